# revision 10
# baseline (speedup 1.0000x reference)
"""Trainium2 Bass kernel for nn_Attention_48687749268214 (v2).

Self-attention with pair-bias: LN(x) -> qkv -> q/k LN -> heads,
bias = einsum('bijc,hc->bhij', LN(pair), w_bias), softmax(qk/8+bias) @ v -> proj.

Sharding: sequence-shard the i axis across 8 cores (64 query rows each),
j rolled per core so own query rows sit at local rows 0:63. No collectives.

v2 design (vs v1 baseline at ~500us):
 - pair ships from host PRE-TRANSPOSED [C, NIJ] in bf16: halves HBM traffic
   and eliminates all on-device pair transposes + their psum drains.
 - bias matmul runs directly on the transposed panels: out[h, j] rows.
   wg14 col 12 = ones harvests 768*mu; col 13 = 0 reserves the sumsq row.
 - per-ij sumsq (for pair-LN variance) via elementwise squares split
   DVE/ACT, reduced over c by a ones-column (e14) matmul accumulating into
   row 13 of the same bias psum -> drained together, no extra drains.
 - bias + e13 matmuls issue as TWO concurrent 32-aligned column strips
   (M=14 at psum rows 0:14 and 32:46) -> ~2x PE throughput; the two strip
   partial-sums are merged once in the attention phase (cheap), not per panel.
 - x-path (LN, qkv, q/k LN, kT/qT) is sliced into small pieces interleaved
   into the pair-panel loop so PE/DVE/ACT stay busy while DMA streams pair.
"""

import sys

sys.path.insert(0, "/opt/trn_rl_repo")

from contextlib import ExitStack

import ml_dtypes
import numpy as np

import concourse.bass as bass
import concourse.tile as tile
from concourse import bacc, mybir
from concourse.bass_utils import run_bass_kernel_spmd
from concourse.masks import make_identity

F32 = mybir.dt.float32
FP8 = mybir.dt.float8e4
BF16 = mybir.dt.bfloat16
AF = mybir.ActivationFunctionType
OP = mybir.AluOpType

C = 768
H = 12
HD = 64
N = 512
NCORES = 8
IB = N // NCORES  # 64 i rows per core
NIJ = IB * N  # 32768 pair rows per core
NPNL = IB  # 64 panels: panel p = i-row p, all 512 j
TJ = 1024  # j columns per DMA tile (2 panels)
NT = NPNL // 2  # 32 dma tiles
EPS = 1e-5
RC = 1.0 / C

bf = ml_dtypes.bfloat16


def _build():
    nc = bacc.Bacc(
        "TRN2", target_bir_lowering=False, debug=False, num_devices=NCORES
    )

    pairT_d = nc.dram_tensor("pairT", [C, NIJ], BF16, kind="ExternalInput").ap()
    x_d = nc.dram_tensor("x_s", [N, C], BF16, kind="ExternalInput").ap()
    wqkvt_d = nc.dram_tensor("wqkvt", [C, 3 * C], BF16, kind="ExternalInput").ap()
    bqkv_d = nc.dram_tensor("bqkv", [1, 3 * C], BF16, kind="ExternalInput").ap()
    wprojt_d = nc.dram_tensor("wprojt", [C, C], BF16, kind="ExternalInput").ap()
    bproj_d = nc.dram_tensor("bproj", [1, C], BF16, kind="ExternalInput").ap()
    wg14_d = nc.dram_tensor("wg14", [C, 14], BF16, kind="ExternalInput").ap()
    ncs_d = nc.dram_tensor("ncs", [1, H], BF16, kind="ExternalInput").ap()
    reps_d = nc.dram_tensor("lnreps", [6, C], BF16, kind="ExternalInput").ap()
    out_d = nc.dram_tensor("out", [IB, C], F32, kind="ExternalOutput").ap()

    with tile.TileContext(nc) as tc, ExitStack() as ctx:
        sing = ctx.enter_context(tc.tile_pool(name="sing", bufs=1))
        pairp = ctx.enter_context(tc.tile_pool(name="pairp", bufs=2))
        sqp = ctx.enter_context(tc.tile_pool(name="sqp", bufs=2))
        statp = ctx.enter_context(tc.tile_pool(name="statp", bufs=2))
        stagep = ctx.enter_context(tc.tile_pool(name="stagep", bufs=3))
        attnp = ctx.enter_context(tc.tile_pool(name="attnp", bufs=3))
        ps_pair = ctx.enter_context(tc.tile_pool(name="ps_pair", bufs=2, space="PSUM"))
        ps_mm = ctx.enter_context(tc.tile_pool(name="ps_mm", bufs=2, space="PSUM"))
        ps_t = ctx.enter_context(tc.tile_pool(name="ps_t", bufs=3, space="PSUM"))

        # ---- singles ----
        id128 = sing.tile([128, 128], BF16)
        make_identity(nc, id128)
        ones_col = sing.tile([1, 128], BF16)
        nc.vector.memset(ones_col, 1.0)
        e14 = sing.tile([128, 14], BF16)
        nc.vector.memset(e14, 0.0)
        nc.vector.memset(e14[:, 13:14], 1.0)
        e14f8 = sing.tile([128, 14], FP8)
        nc.vector.memset(e14f8, 0.0)
        nc.vector.memset(e14f8[:, 13:14], 1.0)

        wg14 = sing.tile([128, 6, 14], BF16)
        nc.sync.dma_start(out=wg14, in_=wg14_d.rearrange("(k p) o -> p k o", p=128))

        # bias_r: [i, 28, j]; rows 0:14 strip-A partial, 14:28 strip-B partial
        # (rows within strip: 0-11 heads, 12 = 768*mu, 13 = sumsq).
        # merge_bias_half folds B into A in place: rows 0:14 become the total.
        bias_r = sing.tile([IB, 28, N], BF16)
        rmap = sing.tile([IB, N], BF16)
        ncsrep = sing.tile([IB, H], BF16)

        # x-path singles
        wqkvt = sing.tile([128, 6, 3 * C], BF16)
        bqkv = sing.tile([1, 3 * C], BF16)
        reps = sing.tile([128, 6, C], BF16)
        xn = sing.tile([128, 4, C], BF16)
        xnT = sing.tile([128, 6, N], BF16)
        qkv = sing.tile([128, 4, 3 * C], BF16)
        kT = sing.tile([128, 6, N], BF16)
        qT = sing.tile([128, 6, IB], BF16)
        wprojt = sing.tile([128, 6, C], BF16)
        bproj = sing.tile([1, C], BF16)

        wq_r = wqkvt_d.rearrange("(k p) o -> p k o", p=128)

        # ================== x-path pieces ==================
        xstat = {}

        def x_dma_initial():
            nc.sync.dma_start(out=xn, in_=x_d.rearrange("(t p) c -> p t c", p=128))
            nc.sync.dma_start(out=bqkv, in_=bqkv_d)
            for rI in range(6):
                nc.gpsimd.dma_start(
                    out=reps[:, rI, :],
                    in_=bass.AP(
                        tensor=reps_d.tensor, offset=rI * C, ap=[[0, 128], [1, C]]
                    ),
                )
            nc.gpsimd.dma_start(
                out=ncsrep,
                in_=bass.AP(tensor=ncs_d.tensor, offset=0, ap=[[0, IB], [1, H]]),
            )

        def x_ln_stats():
            bnx = statp.tile([128, 8, 6], F32, tag="bnx", bufs=1)
            mvx = statp.tile([128, 4, 2], F32, tag="mvx", bufs=1)
            rxx = statp.tile([128, 4], F32, tag="rx", bufs=1)
            xv = xn.rearrange("p t (k c) -> p t k c", k=2)
            for t in range(4):
                for k in range(2):
                    nc.vector.bn_stats(out=bnx[:, 2 * t + k, :], in_=xv[:, t, k, :])
            for t in range(4):
                nc.vector.bn_aggr(out=mvx[:, t, :], in_=bnx[:, 2 * t : 2 * t + 2, :])
            nc.vector.tensor_scalar(
                out=rxx, in0=mvx[:, :, 1], scalar1=EPS, scalar2=None, op0=OP.add
            )
            nc.vector.reciprocal(out=rxx, in_=rxx)
            nc.scalar.activation(out=rxx, in_=rxx, func=AF.Sqrt)
            xstat["mvx"] = mvx
            xstat["rxx"] = rxx

        def x_ln_apply(ts0, ts1):
            mvx, rxx = xstat["mvx"], xstat["rxx"]
            for t in (ts0, ts1):
                nc.vector.tensor_scalar(
                    out=xn[:, t, :], in0=xn[:, t, :],
                    scalar1=mvx[:, t, 0:1], scalar2=rxx[:, t : t + 1],
                    op0=OP.subtract, op1=OP.mult,
                )
                nc.vector.tensor_tensor(
                    out=xn[:, t, :], in0=xn[:, t, :], in1=reps[:, 0, :], op=OP.mult
                )
                nc.vector.tensor_tensor(
                    out=xn[:, t, :], in0=xn[:, t, :], in1=reps[:, 1, :], op=OP.add
                )

        def x_transpose(ch):
            pst = ps_t.tile([128, N], BF16, tag="pst")
            for t in range(4):
                nc.tensor.transpose(
                    pst[:, t * 128 : (t + 1) * 128],
                    xn[:, t, ch * 128 : (ch + 1) * 128],
                    id128,
                )
            nc.vector.tensor_copy(out=xnT[:, ch, :], in_=pst)

        QKV_OCH = [(0, 512), (512, 512), (1024, 512), (1536, 512), (2048, 256)]

        def x_qkv_piece(idx):
            t, oi = idx // 5, idx % 5
            occ, ocs = QKV_OCH[oi]
            pmm = ps_mm.tile([128, N], F32, tag="mm")
            for ch in range(6):
                nc.tensor.matmul(
                    pmm[:, 0:ocs],
                    lhsT=xnT[:, ch, t * 128 : (t + 1) * 128],
                    rhs=wqkvt[:, ch, occ : occ + ocs],
                    start=(ch == 0), stop=False,
                )
            nc.tensor.matmul(
                pmm[:, 0:ocs], lhsT=ones_col[:, 0:128],
                rhs=bqkv[:, occ : occ + ocs], start=False, stop=True,
            )
            if idx % 2 == 0:
                nc.vector.tensor_copy(out=qkv[:, t, occ : occ + ocs], in_=pmm[:, 0:ocs])
            else:
                nc.scalar.copy(out=qkv[:, t, occ : occ + ocs], in_=pmm[:, 0:ocs])

        def qk_ln_stats1():
            bnq = statp.tile([128, 16, 6], F32, tag="bnq", bufs=1)
            qv = qkv[:, :, 0 : 2 * C].rearrange("p t (k c) -> p t k c", k=4)
            for t in range(4):
                for k in range(4):
                    nc.vector.bn_stats(out=bnq[:, 4 * t + k, :], in_=qv[:, t, k, :])
            xstat["bnq"] = bnq

        def qk_ln_stats2():
            bnq = xstat["bnq"]
            mvq = statp.tile([128, 8, 2], F32, tag="mvq", bufs=1)
            rqq = statp.tile([128, 8], F32, tag="rq", bufs=1)
            for col in range(8):
                nc.vector.bn_aggr(out=mvq[:, col, :], in_=bnq[:, 2 * col : 2 * col + 2, :])
            nc.vector.tensor_scalar(
                out=rqq, in0=mvq[:, :, 1], scalar1=EPS, scalar2=None, op0=OP.add
            )
            nc.vector.reciprocal(out=rqq, in_=rqq)
            nc.scalar.activation(out=rqq, in_=rqq, func=AF.Sqrt)
            xstat["mvq"] = mvq
            xstat["rqq"] = rqq

        def qk_ln_apply(t, qi):
            mvq, rqq = xstat["mvq"], xstat["rqq"]
            off = qi * C
            col = t * 2 + qi
            gr = 2 + 2 * qi
            nc.vector.tensor_scalar(
                out=qkv[:, t, off : off + C], in0=qkv[:, t, off : off + C],
                scalar1=mvq[:, col, 0:1], scalar2=rqq[:, col : col + 1],
                op0=OP.subtract, op1=OP.mult,
            )
            nc.vector.tensor_tensor(
                out=qkv[:, t, off : off + C], in0=qkv[:, t, off : off + C],
                in1=reps[:, gr, :], op=OP.mult,
            )
            nc.vector.tensor_tensor(
                out=qkv[:, t, off : off + C], in0=qkv[:, t, off : off + C],
                in1=reps[:, gr + 1, :], op=OP.add,
            )

        def k_transpose(ch):
            pst = ps_t.tile([128, N], BF16, tag="pst")
            for t in range(4):
                nc.tensor.transpose(
                    pst[:, t * 128 : (t + 1) * 128],
                    qkv[:, t, C + ch * 128 : C + (ch + 1) * 128],
                    id128,
                )
            nc.vector.tensor_copy(out=kT[:, ch, :], in_=pst)

        def q_transpose():
            pst = ps_t.tile([128, N], BF16, tag="pst")
            for ch in range(6):
                nc.tensor.transpose(
                    pst[:, ch * IB : (ch + 1) * IB],
                    qkv[0:IB, 0, ch * 128 : (ch + 1) * 128],
                    id128[0:IB, 0:IB],
                )
            nc.vector.tensor_copy(
                out=qT.rearrange("p a b -> p (a b)"), in_=pst[:, 0 : 6 * IB]
            )

        # panel -> list of closures
        sched = {}

        def at(p, fn):
            sched.setdefault(p, []).append(fn)

        at(2, x_ln_stats)
        at(3, lambda: x_ln_apply(0, 1))
        at(4, lambda: x_ln_apply(2, 3))
        for ch in range(6):
            at(5 + ch, lambda ch=ch: x_transpose(ch))
        for idx in range(20):
            at(32 + idx, lambda idx=idx: x_qkv_piece(idx))
        at(53, qk_ln_stats1)
        at(54, qk_ln_stats2)
        for i, (t, qi) in enumerate([(t, qi) for t in range(4) for qi in range(2)]):
            at(55 + i // 2, lambda t=t, qi=qi: qk_ln_apply(t, qi))
        at(59, lambda: k_transpose(0))
        at(60, lambda: k_transpose(1))
        at(60, lambda: k_transpose(2))
        at(61, lambda: k_transpose(3))
        at(61, lambda: k_transpose(4))
        at(62, lambda: k_transpose(5))
        at(63, q_transpose)

        # ================== maps ==================
        va = sing.tile([IB, N], F32)
        tsm = sing.tile([IB, N], BF16)
        sqm = sing.tile([IB, N], BF16)

        def merge_bias_half(b):
            r = slice(32 * b, 32 * b + 32)
            nc.vector.tensor_tensor(
                out=bias_r[r, 0:14, :].rearrange("p a b -> p (a b)"),
                in0=bias_r[r, 0:14, :].rearrange("p a b -> p (a b)"),
                in1=bias_r[r, 14:28, :].rearrange("p a b -> p (a b)"),
                op=OP.add,
            )

        def emit_maps_half(b):
            r = slice(32 * b, 32 * b + 32)
            # merged bias_r row 12 = 768*mu, row 13 = sumsq
            nc.vector.tensor_scalar(
                out=tsm[r], in0=bias_r[r, 12, :], scalar1=RC, scalar2=None, op0=OP.mult
            )
            nc.gpsimd.tensor_tensor(out=sqm[r], in0=tsm[r], in1=tsm[r], op=OP.mult)
            nc.vector.scalar_tensor_tensor(
                out=va[r], in0=bias_r[r, 13, :], scalar=RC,
                in1=sqm[r], op0=OP.mult, op1=OP.subtract,
            )
            nc.vector.tensor_scalar(
                out=va[r], in0=va[r], scalar1=EPS, scalar2=None, op0=OP.add
            )
            nc.vector.reciprocal(out=va[r], in_=va[r])
            nc.scalar.activation(out=rmap[r], in_=va[r], func=AF.Sqrt)

        # ================== pair panel loop ==================
        pv = pairT_d.rearrange("(k p) (n j) -> n p k j", p=128, j=TJ)
        x_dma_initial()

        STRIP_CH = ((0, 2, 4), (1, 3, 5))  # chunks per column strip

        for ti in range(NT):
            grp = pairp.tile([128, 6, TJ], BF16, tag="grp")
            nc.sync.dma_start(out=grp, in_=pv[ti])
            if 1 <= ti <= 6:
                k = ti - 1
                nc.sync.dma_start(out=wqkvt[:, k, :], in_=wq_r[:, k, :])
            if ti == 7:
                nc.sync.dma_start(
                    out=wprojt, in_=wprojt_d.rearrange("(k p) o -> p k o", p=128)
                )
                nc.sync.dma_start(out=bproj, in_=bproj_d)

            # squares: DVE chunks 0:3 (bf16), ACT chunks 3:6 (fp8 out)
            sqD = sqp.tile([128, 3, TJ], BF16, tag="sqD")
            nc.vector.tensor_tensor(
                out=sqD.rearrange("p a b -> p (a b)"),
                in0=grp[:, 0:3, :].rearrange("p a b -> p (a b)"),
                in1=grp[:, 0:3, :].rearrange("p a b -> p (a b)"),
                op=OP.mult,
            )
            sqA = sqp.tile([128, 3, TJ], FP8, tag="sqA")
            nc.scalar.activation(
                out=sqA.rearrange("p a b -> p (a b)"),
                in_=grp[:, 3:6, :].rearrange("p a b -> p (a b)"),
                func=AF.Square,
            )
            # DVE partial: aD2 = sqD0 + sqD1 + sqD2 (second add in place)
            aD2 = sqp.tile([128, TJ], BF16, tag="aD2")
            nc.vector.tensor_tensor(
                out=aD2, in0=sqD[:, 0, :], in1=sqD[:, 1, :], op=OP.add
            )
            nc.vector.tensor_tensor(
                out=aD2, in0=aD2, in1=sqD[:, 2, :], op=OP.add
            )

            for s in range(2):
                pnl = 2 * ti + s
                js = slice(s * N, (s + 1) * N)
                bps = ps_pair.tile([128, N], F32, tag="bps")
                # two concurrent column strips (psum rows 0:14 / 32:46),
                # interleaved in program order so they stream concurrently
                for cc in range(3):
                    for g, chs in enumerate(STRIP_CH):
                        nc.tensor.matmul(
                            bps[32 * g : 32 * g + 14, :],
                            lhsT=wg14[:, chs[cc], :],
                            rhs=grp[:, chs[cc], js],
                            start=(cc == 0), stop=False,
                        )
                # sumsq rows: strip A gets aD2 + sqA0; strip B gets sqA1 + sqA2
                nc.tensor.matmul(
                    bps[0:14, :], lhsT=e14, rhs=aD2[:, js],
                    start=False, stop=False,
                )
                nc.tensor.matmul(
                    bps[32:46, :], lhsT=e14f8, rhs=sqA[:, 1, js],
                    start=False, stop=False,
                )
                nc.tensor.matmul(
                    bps[0:14, :], lhsT=e14f8, rhs=sqA[:, 0, js],
                    start=False, stop=True,
                )
                nc.tensor.matmul(
                    bps[32:46, :], lhsT=e14f8, rhs=sqA[:, 2, js],
                    start=False, stop=True,
                )
                stg = stagep.tile([46, N], BF16, tag="stg")
                nc.scalar.copy(out=stg, in_=bps[0:46, :])
                nc.gpsimd.dma_start(
                    out=bias_r[pnl : pnl + 1, 0:14, :], in_=stg[0:14, :]
                )
                nc.gpsimd.dma_start(
                    out=bias_r[pnl : pnl + 1, 14:28, :], in_=stg[32:46, :]
                )

                for fn in sched.get(pnl, ()):
                    fn()
                if pnl == 34:
                    merge_bias_half(0)
                    emit_maps_half(0)

        merge_bias_half(1)
        emit_maps_half(1)

        # ================== attention ==================
        o_sb = sing.tile([IB, H, HD], BF16)
        bfull = sing.tile([IB, H, N], BF16)

        def emit_bfix(h):
            t1 = attnp.tile([IB, N], BF16, tag="t1")
            nc.vector.scalar_tensor_tensor(
                out=t1, in0=bias_r[:, 12, :], scalar=ncsrep[:, h : h + 1],
                in1=bias_r[:, h, :], op0=OP.mult, op1=OP.add,
            )
            nc.gpsimd.tensor_tensor(
                out=bfull[:, h, :], in0=t1, in1=rmap, op=OP.mult
            )

        def emit_qk(h):
            sps = ps_pair.tile([IB, N], F32, tag="bps")
            bp = (h % 2) * 64
            nc.tensor.matmul(
                sps[0:IB, :],
                lhsT=qT[bp : bp + 64, h // 2, :],
                rhs=kT[bp : bp + 64, h // 2, :],
                start=True, stop=True,
            )
            return sps

        emit_bfix(0)
        emit_bfix(1)
        sps_next = emit_qk(0)
        pps_o0 = ps_mm.tile([128, N], F32, tag="mm")
        pps_o1 = ps_mm.tile([128, N], F32, tag="mm")
        pps_out = [pps_o0, pps_o1]
        for h in range(H):
            if h + 2 < H:
                emit_bfix(h + 2)
            sps = sps_next
            if h + 1 < H:
                sps_next = emit_qk(h + 1)
            sim = attnp.tile([IB, N], F32, tag="sim")
            nc.vector.scalar_tensor_tensor(
                out=sim, in0=sps[0:IB, :], scalar=0.125, in1=bfull[:, h, :],
                op0=OP.mult, op1=OP.add,
            )
            # logits are small (LN'd q/k, tiny weights): exp without max-sub
            den = attnp.tile([IB, 1], F32, tag="den")
            nc.scalar.activation(out=sim, in_=sim, func=AF.Exp, accum_out=den)
            nc.vector.reciprocal(out=den, in_=den)
            attn = attnp.tile([IB, N], BF16, tag="attn")
            nc.vector.tensor_scalar(
                out=attn, in0=sim, scalar1=den, scalar2=None, op0=OP.mult
            )
            aps = ps_t.tile([128, N], BF16, tag="pst")
            for jc in range(4):
                nc.tensor.transpose(
                    aps[:, jc * IB : (jc + 1) * IB],
                    attn[:, jc * 128 : (jc + 1) * 128],
                    id128[0:IB, 0:IB],
                )
            aT = attnp.tile([128, 4, IB], BF16, tag="aT")
            nc.scalar.copy(
                out=aT.rearrange("p a b -> p (a b)"), in_=aps[:, 0 : 4 * IB]
            )
            ops = ps_t.tile([IB, N], F32, tag="pst")
            for jc in range(4):
                nc.tensor.matmul(
                    ops[0:IB, 0:HD],
                    lhsT=aT[:, jc, :],
                    rhs=qkv[:, jc, 2 * C + h * HD : 2 * C + (h + 1) * HD],
                    start=(jc == 0), stop=(jc == 3),
                )
            nc.scalar.copy(out=o_sb[:, h, :], in_=ops[0:IB, 0:HD])

            # proj c-block ch = heads (2ch, 2ch+1): interleave into head loop
            if h % 2 == 1:
                ch = h // 2
                pso = ps_t.tile([128, N], BF16, tag="pst")
                nc.tensor.transpose(
                    pso[:, 0:IB],
                    o_sb.rearrange("p a b -> p (a b)")[:, ch * 128 : (ch + 1) * 128],
                    id128[0:IB, 0:IB],
                )
                oTc = attnp.tile([128, IB], BF16, tag="oTc")
                nc.scalar.copy(out=oTc, in_=pso[:, 0:IB])
                for oi, (occ, ocs) in enumerate([(0, 512), (512, 256)]):
                    nc.tensor.matmul(
                        pps_out[oi][0:IB, 0:ocs],
                        lhsT=oTc[:, 0:IB],
                        rhs=wprojt[:, ch, occ : occ + ocs],
                        start=(ch == 0), stop=False,
                    )

        # ================== output proj epilogue ==================
        out_sb = sing.tile([IB, C], F32)
        for oi, (occ, ocs) in enumerate([(0, 512), (512, 256)]):
            nc.tensor.matmul(
                pps_out[oi][0:IB, 0:ocs], lhsT=ones_col[:, 0:IB],
                rhs=bproj[:, occ : occ + ocs], start=False, stop=True,
            )
            nc.vector.tensor_copy(
                out=out_sb[:, occ : occ + ocs], in_=pps_out[oi][0:IB, 0:ocs]
            )
        nc.sync.dma_start(out=out_d, in_=out_sb)

    nc.compile()
    return nc


_NC = None
_LAST_MAPS = None


def prep_maps(x, pair, ln_g, ln_b, w_qkv, b_qkv, w_proj, b_proj, w_bias,
              pn_g, pn_b, qln_g, qln_b, kln_g, kln_b):
    x = np.asarray(x, np.float32)
    pair = np.asarray(pair, np.float32)
    wqkvt = np.ascontiguousarray(np.asarray(w_qkv, np.float32).T).astype(bf)
    wprojt = np.ascontiguousarray(np.asarray(w_proj, np.float32).T).astype(bf)
    wg_host = np.ascontiguousarray(
        (np.asarray(pn_g, np.float32)[:, None] * np.asarray(w_bias, np.float32).T)
    )
    cs = wg_host.sum(axis=0)  # colsum over c, [H]
    wg14 = np.concatenate(
        [wg_host, np.ones((C, 1), np.float32), np.zeros((C, 1), np.float32)],
        axis=1,
    ).astype(bf)  # [C, 14]; col 12 harvests 768*mu, col 13 = sumsq slot
    ncs = (-cs / C)[None].astype(bf)  # [1, H]; scaled: mu arrives as 768*mu
    reps = np.stack(
        [np.asarray(a, np.float32) for a in (ln_g, ln_b, qln_g, qln_b, kln_g, kln_b)]
    ).astype(bf)
    bqkv = np.asarray(b_qkv, np.float32)[None].astype(bf)
    bproj = np.asarray(b_proj, np.float32)[None].astype(bf)

    in_maps = []
    for k in range(NCORES):
        ps = pair[0, k * IB : (k + 1) * IB]  # [64, 512, 768]
        ps = np.roll(ps, -k * IB, axis=1)  # roll j to match rolled x
        pT = np.ascontiguousarray(ps.reshape(NIJ, C).T).astype(bf)  # [C, NIJ]
        xk = np.roll(x[0], -k * IB, axis=0).astype(bf)
        in_maps.append(
            {
                "pairT": pT,
                "x_s": np.ascontiguousarray(xk),
                "wqkvt": wqkvt,
                "bqkv": bqkv,
                "wprojt": wprojt,
                "bproj": bproj,
                "wg14": wg14,
                "ncs": ncs,
                "lnreps": reps,
            }
        )

    return in_maps


def kernel(**inputs):
    global _NC, _LAST_MAPS
    if _NC is None:
        _NC = _build()
    in_maps = prep_maps(**inputs)
    _LAST_MAPS = in_maps
    res = run_bass_kernel_spmd(_NC, in_maps, list(range(NCORES)))
    outs = [res.results[k]["out"] for k in range(NCORES)]
    return np.concatenate(outs, axis=0)[None].astype(np.float32)


# revision 13
# speedup vs baseline: 1.0214x; 1.0214x over previous
"""Trainium2 Bass kernel for nn_Attention_48687749268214 (v2).

Self-attention with pair-bias: LN(x) -> qkv -> q/k LN -> heads,
bias = einsum('bijc,hc->bhij', LN(pair), w_bias), softmax(qk/8+bias) @ v -> proj.

Sharding: sequence-shard the i axis across 8 cores (64 query rows each),
j rolled per core so own query rows sit at local rows 0:63. No collectives.

v2 design (vs v1 baseline at ~500us):
 - pair ships from host PRE-TRANSPOSED [C, NIJ] in bf16: halves HBM traffic
   and eliminates all on-device pair transposes + their psum drains.
 - bias matmul runs directly on the transposed panels: out[h, j] rows.
   wg14 col 12 = ones harvests 768*mu; col 13 = 0 reserves the sumsq row.
 - per-ij sumsq (for pair-LN variance) via elementwise squares split
   DVE/ACT, reduced over c by a ones-column (e14) matmul accumulating into
   row 13 of the same bias psum -> drained together, no extra drains.
 - bias + e13 matmuls issue as TWO concurrent 32-aligned column strips
   (M=14 at psum rows 0:14 and 32:46) -> ~2x PE throughput; the two strip
   partial-sums are merged once in the attention phase (cheap), not per panel.
 - x-path (LN, qkv, q/k LN, kT/qT) is sliced into small pieces interleaved
   into the pair-panel loop so PE/DVE/ACT stay busy while DMA streams pair.
"""

import sys

sys.path.insert(0, "/opt/trn_rl_repo")

from contextlib import ExitStack

import ml_dtypes
import numpy as np

import concourse.bass as bass
import concourse.tile as tile
from concourse import bacc, mybir
from concourse.bass_utils import run_bass_kernel_spmd
from concourse.masks import make_identity

F32 = mybir.dt.float32
FP8 = mybir.dt.float8e4
BF16 = mybir.dt.bfloat16
AF = mybir.ActivationFunctionType
OP = mybir.AluOpType

C = 768
H = 12
HD = 64
N = 512
NCORES = 8
IB = N // NCORES  # 64 i rows per core
NIJ = IB * N  # 32768 pair rows per core
NPNL = IB  # 64 panels: panel p = i-row p, all 512 j
TJ = 1024  # j columns per DMA tile (2 panels)
NT = NPNL // 2  # 32 dma tiles
EPS = 1e-5
RC = 1.0 / C

bf = ml_dtypes.bfloat16


def _build():
    nc = bacc.Bacc(
        "TRN2", target_bir_lowering=False, debug=False, num_devices=NCORES
    )

    pairT_d = nc.dram_tensor("pairT", [C, NIJ], BF16, kind="ExternalInput").ap()
    x_d = nc.dram_tensor("x_s", [N, C], BF16, kind="ExternalInput").ap()
    wqkvt_d = nc.dram_tensor("wqkvt", [C, 3 * C], BF16, kind="ExternalInput").ap()
    bqkv_d = nc.dram_tensor("bqkv", [1, 3 * C], BF16, kind="ExternalInput").ap()
    wprojt_d = nc.dram_tensor("wprojt", [C, C], BF16, kind="ExternalInput").ap()
    bproj_d = nc.dram_tensor("bproj", [1, C], BF16, kind="ExternalInput").ap()
    wg14_d = nc.dram_tensor("wg14", [C, 14], BF16, kind="ExternalInput").ap()
    ncs_d = nc.dram_tensor("ncs", [1, H], BF16, kind="ExternalInput").ap()
    reps_d = nc.dram_tensor("lnreps", [6, C], BF16, kind="ExternalInput").ap()
    out_d = nc.dram_tensor("out", [IB, C], F32, kind="ExternalOutput").ap()

    with tile.TileContext(nc) as tc, ExitStack() as ctx:
        sing = ctx.enter_context(tc.tile_pool(name="sing", bufs=1))
        pairp = ctx.enter_context(tc.tile_pool(name="pairp", bufs=2))
        sqp = ctx.enter_context(tc.tile_pool(name="sqp", bufs=2))
        statp = ctx.enter_context(tc.tile_pool(name="statp", bufs=2))
        stagep = ctx.enter_context(tc.tile_pool(name="stagep", bufs=3))
        attnp = ctx.enter_context(tc.tile_pool(name="attnp", bufs=3))
        ps_pair = ctx.enter_context(tc.tile_pool(name="ps_pair", bufs=4, space="PSUM"))
        ps_mm = ctx.enter_context(tc.tile_pool(name="ps_mm", bufs=2, space="PSUM"))
        ps_t = ctx.enter_context(tc.tile_pool(name="ps_t", bufs=2, space="PSUM"))

        # ---- singles ----
        id128 = sing.tile([128, 128], BF16)
        make_identity(nc, id128)
        ones_col = sing.tile([1, 128], BF16)
        nc.vector.memset(ones_col, 1.0)
        e14 = sing.tile([128, 14], BF16)
        nc.vector.memset(e14, 0.0)
        nc.vector.memset(e14[:, 13:14], 1.0)
        e14f8 = sing.tile([128, 14], FP8)
        nc.vector.memset(e14f8, 0.0)
        nc.vector.memset(e14f8[:, 13:14], 1.0)

        wg14 = sing.tile([128, 6, 14], BF16)
        nc.sync.dma_start(out=wg14, in_=wg14_d.rearrange("(k p) o -> p k o", p=128))

        # bias_r: [i, 28, j]; rows 0:14 strip-A partial, 14:28 strip-B partial
        # (rows within strip: 0-11 heads, 12 = 768*mu, 13 = sumsq).
        # merge_bias_half folds B into A in place: rows 0:14 become the total.
        bias_r = sing.tile([IB, 28, N], BF16)
        rmap = sing.tile([IB, N], BF16)
        ncsrep = sing.tile([IB, H], BF16)

        # x-path singles
        wqkvt = sing.tile([128, 6, 3 * C], BF16)
        bqkv = sing.tile([1, 3 * C], BF16)
        reps = sing.tile([128, 6, C], BF16)
        xn = sing.tile([128, 4, C], BF16)
        xnT = sing.tile([128, 6, N], BF16)
        qkv = sing.tile([128, 4, 3 * C], BF16)
        kT = sing.tile([128, 6, N], BF16)
        qT = sing.tile([128, 6, IB], BF16)
        wprojt = sing.tile([128, 6, C], BF16)
        bproj = sing.tile([1, C], BF16)

        wq_r = wqkvt_d.rearrange("(k p) o -> p k o", p=128)

        # ================== x-path pieces ==================
        xstat = {}

        def x_dma_initial():
            nc.sync.dma_start(out=xn, in_=x_d.rearrange("(t p) c -> p t c", p=128))
            nc.sync.dma_start(out=bqkv, in_=bqkv_d)
            for rI in range(6):
                nc.gpsimd.dma_start(
                    out=reps[:, rI, :],
                    in_=bass.AP(
                        tensor=reps_d.tensor, offset=rI * C, ap=[[0, 128], [1, C]]
                    ),
                )
            nc.gpsimd.dma_start(
                out=ncsrep,
                in_=bass.AP(tensor=ncs_d.tensor, offset=0, ap=[[0, IB], [1, H]]),
            )

        def x_ln_stats():
            bnx = statp.tile([128, 8, 6], F32, tag="bnx", bufs=1)
            mvx = statp.tile([128, 4, 2], F32, tag="mvx", bufs=1)
            rxx = statp.tile([128, 4], F32, tag="rx", bufs=1)
            xv = xn.rearrange("p t (k c) -> p t k c", k=2)
            for t in range(4):
                for k in range(2):
                    nc.vector.bn_stats(out=bnx[:, 2 * t + k, :], in_=xv[:, t, k, :])
            for t in range(4):
                nc.vector.bn_aggr(out=mvx[:, t, :], in_=bnx[:, 2 * t : 2 * t + 2, :])
            nc.vector.tensor_scalar(
                out=rxx, in0=mvx[:, :, 1], scalar1=EPS, scalar2=None, op0=OP.add
            )
            nc.vector.reciprocal(out=rxx, in_=rxx)
            nc.scalar.activation(out=rxx, in_=rxx, func=AF.Sqrt)
            xstat["mvx"] = mvx
            xstat["rxx"] = rxx

        def x_ln_apply(ts0, ts1):
            mvx, rxx = xstat["mvx"], xstat["rxx"]
            for t in (ts0, ts1):
                nc.vector.tensor_scalar(
                    out=xn[:, t, :], in0=xn[:, t, :],
                    scalar1=mvx[:, t, 0:1], scalar2=rxx[:, t : t + 1],
                    op0=OP.subtract, op1=OP.mult,
                )
                nc.vector.tensor_tensor(
                    out=xn[:, t, :], in0=xn[:, t, :], in1=reps[:, 0, :], op=OP.mult
                )
                nc.vector.tensor_tensor(
                    out=xn[:, t, :], in0=xn[:, t, :], in1=reps[:, 1, :], op=OP.add
                )

        def x_transpose(ch):
            pst = ps_t.tile([128, N], BF16, tag="pst")
            for t in range(4):
                nc.tensor.transpose(
                    pst[:, t * 128 : (t + 1) * 128],
                    xn[:, t, ch * 128 : (ch + 1) * 128],
                    id128,
                )
            nc.vector.tensor_copy(out=xnT[:, ch, :], in_=pst)

        QKV_OCH = [(0, 512), (512, 512), (1024, 512), (1536, 512), (2048, 256)]

        def x_qkv_piece(idx):
            t, oi = idx // 5, idx % 5
            occ, ocs = QKV_OCH[oi]
            pmm = ps_mm.tile([128, N], F32, tag="mm")
            for ch in range(6):
                nc.tensor.matmul(
                    pmm[:, 0:ocs],
                    lhsT=xnT[:, ch, t * 128 : (t + 1) * 128],
                    rhs=wqkvt[:, ch, occ : occ + ocs],
                    start=(ch == 0), stop=False,
                )
            nc.tensor.matmul(
                pmm[:, 0:ocs], lhsT=ones_col[:, 0:128],
                rhs=bqkv[:, occ : occ + ocs], start=False, stop=True,
            )
            if idx % 2 == 0:
                nc.vector.tensor_copy(out=qkv[:, t, occ : occ + ocs], in_=pmm[:, 0:ocs])
            else:
                nc.scalar.copy(out=qkv[:, t, occ : occ + ocs], in_=pmm[:, 0:ocs])

        def qk_ln_stats1():
            bnq = statp.tile([128, 16, 6], F32, tag="bnq", bufs=1)
            qv = qkv[:, :, 0 : 2 * C].rearrange("p t (k c) -> p t k c", k=4)
            for t in range(4):
                for k in range(4):
                    nc.vector.bn_stats(out=bnq[:, 4 * t + k, :], in_=qv[:, t, k, :])
            xstat["bnq"] = bnq

        def qk_ln_stats2():
            bnq = xstat["bnq"]
            mvq = statp.tile([128, 8, 2], F32, tag="mvq", bufs=1)
            rqq = statp.tile([128, 8], F32, tag="rq", bufs=1)
            for col in range(8):
                nc.vector.bn_aggr(out=mvq[:, col, :], in_=bnq[:, 2 * col : 2 * col + 2, :])
            nc.vector.tensor_scalar(
                out=rqq, in0=mvq[:, :, 1], scalar1=EPS, scalar2=None, op0=OP.add
            )
            nc.vector.reciprocal(out=rqq, in_=rqq)
            nc.scalar.activation(out=rqq, in_=rqq, func=AF.Sqrt)
            xstat["mvq"] = mvq
            xstat["rqq"] = rqq

        def qk_ln_apply(t, qi):
            mvq, rqq = xstat["mvq"], xstat["rqq"]
            off = qi * C
            col = t * 2 + qi
            gr = 2 + 2 * qi
            nc.vector.tensor_scalar(
                out=qkv[:, t, off : off + C], in0=qkv[:, t, off : off + C],
                scalar1=mvq[:, col, 0:1], scalar2=rqq[:, col : col + 1],
                op0=OP.subtract, op1=OP.mult,
            )
            nc.vector.tensor_tensor(
                out=qkv[:, t, off : off + C], in0=qkv[:, t, off : off + C],
                in1=reps[:, gr, :], op=OP.mult,
            )
            nc.vector.tensor_tensor(
                out=qkv[:, t, off : off + C], in0=qkv[:, t, off : off + C],
                in1=reps[:, gr + 1, :], op=OP.add,
            )

        def k_transpose(ch):
            pst = ps_t.tile([128, N], BF16, tag="pst")
            for t in range(4):
                nc.tensor.transpose(
                    pst[:, t * 128 : (t + 1) * 128],
                    qkv[:, t, C + ch * 128 : C + (ch + 1) * 128],
                    id128,
                )
            nc.vector.tensor_copy(out=kT[:, ch, :], in_=pst)

        def q_transpose():
            pst = ps_t.tile([128, N], BF16, tag="pst")
            for ch in range(6):
                nc.tensor.transpose(
                    pst[:, ch * IB : (ch + 1) * IB],
                    qkv[0:IB, 0, ch * 128 : (ch + 1) * 128],
                    id128[0:IB, 0:IB],
                )
            nc.vector.tensor_copy(
                out=qT.rearrange("p a b -> p (a b)"), in_=pst[:, 0 : 6 * IB]
            )

        # panel -> list of closures
        sched = {}

        def at(p, fn):
            sched.setdefault(p, []).append(fn)

        at(2, x_ln_stats)
        at(3, lambda: x_ln_apply(0, 1))
        at(4, lambda: x_ln_apply(2, 3))
        for ch in range(6):
            at(5 + ch, lambda ch=ch: x_transpose(ch))
        for idx in range(20):
            at(32 + idx, lambda idx=idx: x_qkv_piece(idx))
        at(53, qk_ln_stats1)
        at(54, qk_ln_stats2)
        for i, (t, qi) in enumerate([(t, qi) for t in range(4) for qi in range(2)]):
            at(55 + i // 2, lambda t=t, qi=qi: qk_ln_apply(t, qi))
        at(59, lambda: k_transpose(0))
        at(60, lambda: k_transpose(1))
        at(60, lambda: k_transpose(2))
        at(61, lambda: k_transpose(3))
        at(61, lambda: k_transpose(4))
        at(62, lambda: k_transpose(5))
        at(63, q_transpose)

        # ================== maps ==================
        va = sing.tile([IB, N], F32)
        tsm = sing.tile([IB, N], BF16)
        sqm = sing.tile([IB, N], BF16)

        def merge_bias_half(b):
            r = slice(32 * b, 32 * b + 32)
            nc.vector.tensor_tensor(
                out=bias_r[r, 0:14, :].rearrange("p a b -> p (a b)"),
                in0=bias_r[r, 0:14, :].rearrange("p a b -> p (a b)"),
                in1=bias_r[r, 14:28, :].rearrange("p a b -> p (a b)"),
                op=OP.add,
            )

        def emit_maps_half(b):
            r = slice(32 * b, 32 * b + 32)
            # merged bias_r row 12 = 768*mu, row 13 = sumsq
            nc.vector.tensor_scalar(
                out=tsm[r], in0=bias_r[r, 12, :], scalar1=RC, scalar2=None, op0=OP.mult
            )
            nc.gpsimd.tensor_tensor(out=sqm[r], in0=tsm[r], in1=tsm[r], op=OP.mult)
            nc.vector.scalar_tensor_tensor(
                out=va[r], in0=bias_r[r, 13, :], scalar=RC,
                in1=sqm[r], op0=OP.mult, op1=OP.subtract,
            )
            nc.vector.tensor_scalar(
                out=va[r], in0=va[r], scalar1=EPS, scalar2=None, op0=OP.add
            )
            nc.vector.reciprocal(out=va[r], in_=va[r])
            nc.scalar.activation(out=rmap[r], in_=va[r], func=AF.Sqrt)

        # ================== pair panel loop ==================
        pv = pairT_d.rearrange("(k p) (n j) -> n p k j", p=128, j=TJ)
        x_dma_initial()

        STRIP_CH = ((0, 2, 4), (1, 3, 5))  # chunks per column strip

        # panels whose psum still needs draining: list of (bps, pnl)
        pending = []

        def drain_pending():
            # deps (the panels' matmuls) completed a tile ago, so these
            # copies never stall the ACT/DVE queues
            while pending:
                bps_p, pnl_p = pending.pop(0)
                stg = stagep.tile([46, N], BF16, tag="stg")
                if pnl_p % 2 == 0:
                    nc.scalar.copy(out=stg, in_=bps_p[0:46, :])
                else:
                    nc.vector.tensor_copy(out=stg, in_=bps_p[0:46, :])
                nc.gpsimd.dma_start(
                    out=bias_r[pnl_p : pnl_p + 1, 0:14, :], in_=stg[0:14, :]
                )
                nc.gpsimd.dma_start(
                    out=bias_r[pnl_p : pnl_p + 1, 14:28, :], in_=stg[32:46, :]
                )

        for ti in range(NT):
            grp = pairp.tile([128, 6, TJ], BF16, tag="grp")
            nc.sync.dma_start(out=grp, in_=pv[ti])
            if 1 <= ti <= 6:
                k = ti - 1
                nc.sync.dma_start(out=wqkvt[:, k, :], in_=wq_r[:, k, :])
            if ti == 7:
                nc.sync.dma_start(
                    out=wprojt, in_=wprojt_d.rearrange("(k p) o -> p k o", p=128)
                )
                nc.sync.dma_start(out=bproj, in_=bproj_d)
            drain_pending()

            # squares: DVE chunks 0:3 (bf16), ACT chunks 3:6 (fp8 out)
            sqD = sqp.tile([128, 3, TJ], BF16, tag="sqD")
            nc.vector.tensor_tensor(
                out=sqD.rearrange("p a b -> p (a b)"),
                in0=grp[:, 0:3, :].rearrange("p a b -> p (a b)"),
                in1=grp[:, 0:3, :].rearrange("p a b -> p (a b)"),
                op=OP.mult,
            )
            sqA = sqp.tile([128, 3, TJ], FP8, tag="sqA")
            nc.scalar.activation(
                out=sqA.rearrange("p a b -> p (a b)"),
                in_=grp[:, 3:6, :].rearrange("p a b -> p (a b)"),
                func=AF.Square,
            )
            # DVE partial: aD2 = sqD0 + sqD1 + sqD2 (second add in place)
            aD2 = sqp.tile([128, TJ], BF16, tag="aD2")
            nc.vector.tensor_tensor(
                out=aD2, in0=sqD[:, 0, :], in1=sqD[:, 1, :], op=OP.add
            )
            nc.vector.tensor_tensor(
                out=aD2, in0=aD2, in1=sqD[:, 2, :], op=OP.add
            )

            # bias matmuls for both panels first (dep: grp DMA only), then
            # the e13 sumsq rows (dep: squares) -- keeps PE streaming while
            # the squares are still in flight on DVE/ACT
            bps2 = []
            for s in range(2):
                js = slice(s * N, (s + 1) * N)
                bps = ps_pair.tile([128, N], F32, tag="bps")
                bps2.append(bps)
                # two concurrent column strips (psum rows 0:14 / 32:46),
                # interleaved in program order so they stream concurrently
                for cc in range(3):
                    for g, chs in enumerate(STRIP_CH):
                        nc.tensor.matmul(
                            bps[32 * g : 32 * g + 14, :],
                            lhsT=wg14[:, chs[cc], :],
                            rhs=grp[:, chs[cc], js],
                            start=(cc == 0), stop=False,
                        )
            for s in range(2):
                pnl = 2 * ti + s
                js = slice(s * N, (s + 1) * N)
                bps = bps2[s]
                # sumsq rows: strip A gets aD2 + sqA0; strip B gets sqA1 + sqA2
                nc.tensor.matmul(
                    bps[0:14, :], lhsT=e14, rhs=aD2[:, js],
                    start=False, stop=False,
                )
                nc.tensor.matmul(
                    bps[32:46, :], lhsT=e14f8, rhs=sqA[:, 1, js],
                    start=False, stop=False,
                )
                nc.tensor.matmul(
                    bps[0:14, :], lhsT=e14f8, rhs=sqA[:, 0, js],
                    start=False, stop=True,
                )
                nc.tensor.matmul(
                    bps[32:46, :], lhsT=e14f8, rhs=sqA[:, 2, js],
                    start=False, stop=True,
                )
                pending.append((bps, pnl))

                for fn in sched.get(pnl, ()):
                    fn()
                if pnl == 34:
                    merge_bias_half(0)
                    emit_maps_half(0)

        drain_pending()
        merge_bias_half(1)
        emit_maps_half(1)

        # ================== attention ==================
        o_sb = sing.tile([IB, H, HD], BF16)
        bfull = sing.tile([IB, H, N], BF16)

        def emit_bfix(h):
            t1 = attnp.tile([IB, N], BF16, tag="t1")
            nc.vector.scalar_tensor_tensor(
                out=t1, in0=bias_r[:, 12, :], scalar=ncsrep[:, h : h + 1],
                in1=bias_r[:, h, :], op0=OP.mult, op1=OP.add,
            )
            nc.gpsimd.tensor_tensor(
                out=bfull[:, h, :], in0=t1, in1=rmap, op=OP.mult
            )

        def emit_qk(h):
            sps = ps_pair.tile([IB, N], F32, tag="bps")
            bp = (h % 2) * 64
            nc.tensor.matmul(
                sps[0:IB, :],
                lhsT=qT[bp : bp + 64, h // 2, :],
                rhs=kT[bp : bp + 64, h // 2, :],
                start=True, stop=True,
            )
            return sps

        emit_bfix(0)
        emit_bfix(1)
        sps_next = emit_qk(0)
        pps_o0 = ps_mm.tile([128, N], F32, tag="mm")
        pps_o1 = ps_mm.tile([128, N], F32, tag="mm")
        pps_out = [pps_o0, pps_o1]
        for h in range(H):
            if h + 2 < H:
                emit_bfix(h + 2)
            sps = sps_next
            if h + 1 < H:
                sps_next = emit_qk(h + 1)
            sim = attnp.tile([IB, N], F32, tag="sim")
            nc.vector.scalar_tensor_tensor(
                out=sim, in0=sps[0:IB, :], scalar=0.125, in1=bfull[:, h, :],
                op0=OP.mult, op1=OP.add,
            )
            # logits are small (LN'd q/k, tiny weights): exp without max-sub
            den = attnp.tile([IB, 1], F32, tag="den")
            nc.scalar.activation(out=sim, in_=sim, func=AF.Exp, accum_out=den)
            nc.vector.reciprocal(out=den, in_=den)
            attn = attnp.tile([IB, N], BF16, tag="attn")
            nc.vector.tensor_scalar(
                out=attn, in0=sim, scalar1=den, scalar2=None, op0=OP.mult
            )
            aps = ps_t.tile([128, N], BF16, tag="pst")
            for jc in range(4):
                nc.tensor.transpose(
                    aps[:, jc * IB : (jc + 1) * IB],
                    attn[:, jc * 128 : (jc + 1) * 128],
                    id128[0:IB, 0:IB],
                )
            aT = attnp.tile([128, 4, IB], BF16, tag="aT")
            nc.scalar.copy(
                out=aT.rearrange("p a b -> p (a b)"), in_=aps[:, 0 : 4 * IB]
            )
            ops = ps_t.tile([IB, N], F32, tag="pst")
            for jc in range(4):
                nc.tensor.matmul(
                    ops[0:IB, 0:HD],
                    lhsT=aT[:, jc, :],
                    rhs=qkv[:, jc, 2 * C + h * HD : 2 * C + (h + 1) * HD],
                    start=(jc == 0), stop=(jc == 3),
                )
            nc.scalar.copy(out=o_sb[:, h, :], in_=ops[0:IB, 0:HD])

            # proj c-block ch = heads (2ch, 2ch+1): interleave into head loop
            if h % 2 == 1:
                ch = h // 2
                pso = ps_t.tile([128, N], BF16, tag="pst")
                nc.tensor.transpose(
                    pso[:, 0:IB],
                    o_sb.rearrange("p a b -> p (a b)")[:, ch * 128 : (ch + 1) * 128],
                    id128[0:IB, 0:IB],
                )
                oTc = attnp.tile([128, IB], BF16, tag="oTc")
                nc.scalar.copy(out=oTc, in_=pso[:, 0:IB])
                for oi, (occ, ocs) in enumerate([(0, 512), (512, 256)]):
                    nc.tensor.matmul(
                        pps_out[oi][0:IB, 0:ocs],
                        lhsT=oTc[:, 0:IB],
                        rhs=wprojt[:, ch, occ : occ + ocs],
                        start=(ch == 0), stop=False,
                    )

        # ================== output proj epilogue ==================
        out_sb = sing.tile([IB, C], F32)
        for oi, (occ, ocs) in enumerate([(0, 512), (512, 256)]):
            nc.tensor.matmul(
                pps_out[oi][0:IB, 0:ocs], lhsT=ones_col[:, 0:IB],
                rhs=bproj[:, occ : occ + ocs], start=False, stop=True,
            )
            nc.vector.tensor_copy(
                out=out_sb[:, occ : occ + ocs], in_=pps_out[oi][0:IB, 0:ocs]
            )
        nc.sync.dma_start(out=out_d, in_=out_sb)

    nc.compile()
    return nc


_NC = None
_LAST_MAPS = None


def prep_maps(x, pair, ln_g, ln_b, w_qkv, b_qkv, w_proj, b_proj, w_bias,
              pn_g, pn_b, qln_g, qln_b, kln_g, kln_b):
    x = np.asarray(x, np.float32)
    pair = np.asarray(pair, np.float32)
    wqkvt = np.ascontiguousarray(np.asarray(w_qkv, np.float32).T).astype(bf)
    wprojt = np.ascontiguousarray(np.asarray(w_proj, np.float32).T).astype(bf)
    wg_host = np.ascontiguousarray(
        (np.asarray(pn_g, np.float32)[:, None] * np.asarray(w_bias, np.float32).T)
    )
    cs = wg_host.sum(axis=0)  # colsum over c, [H]
    wg14 = np.concatenate(
        [wg_host, np.ones((C, 1), np.float32), np.zeros((C, 1), np.float32)],
        axis=1,
    ).astype(bf)  # [C, 14]; col 12 harvests 768*mu, col 13 = sumsq slot
    ncs = (-cs / C)[None].astype(bf)  # [1, H]; scaled: mu arrives as 768*mu
    reps = np.stack(
        [np.asarray(a, np.float32) for a in (ln_g, ln_b, qln_g, qln_b, kln_g, kln_b)]
    ).astype(bf)
    bqkv = np.asarray(b_qkv, np.float32)[None].astype(bf)
    bproj = np.asarray(b_proj, np.float32)[None].astype(bf)

    in_maps = []
    for k in range(NCORES):
        ps = pair[0, k * IB : (k + 1) * IB]  # [64, 512, 768]
        ps = np.roll(ps, -k * IB, axis=1)  # roll j to match rolled x
        pT = np.ascontiguousarray(ps.reshape(NIJ, C).T).astype(bf)  # [C, NIJ]
        xk = np.roll(x[0], -k * IB, axis=0).astype(bf)
        in_maps.append(
            {
                "pairT": pT,
                "x_s": np.ascontiguousarray(xk),
                "wqkvt": wqkvt,
                "bqkv": bqkv,
                "wprojt": wprojt,
                "bproj": bproj,
                "wg14": wg14,
                "ncs": ncs,
                "lnreps": reps,
            }
        )

    return in_maps


def kernel(**inputs):
    global _NC, _LAST_MAPS
    if _NC is None:
        _NC = _build()
    in_maps = prep_maps(**inputs)
    _LAST_MAPS = in_maps
    res = run_bass_kernel_spmd(_NC, in_maps, list(range(NCORES)))
    outs = [res.results[k]["out"] for k in range(NCORES)]
    return np.concatenate(outs, axis=0)[None].astype(np.float32)


# revision 17
# speedup vs baseline: 1.1665x; 1.1421x over previous
"""Trainium2 Bass kernel for nn_Attention_48687749268214 (v2).

Self-attention with pair-bias: LN(x) -> qkv -> q/k LN -> heads,
bias = einsum('bijc,hc->bhij', LN(pair), w_bias), softmax(qk/8+bias) @ v -> proj.

Sharding: sequence-shard the i axis across 8 cores (64 query rows each),
j rolled per core so own query rows sit at local rows 0:63. No collectives.

v2 design (vs v1 baseline at ~500us):
 - pair ships from host PRE-TRANSPOSED [C, NIJ] in bf16: halves HBM traffic
   and eliminates all on-device pair transposes + their psum drains.
 - bias matmul runs directly on the transposed panels: out[h, j] rows.
   wg14 col 12 = ones harvests 768*mu; col 13 = 0 reserves the sumsq row.
 - per-ij sumsq (for pair-LN variance) via elementwise squares split
   DVE/ACT, reduced over c by a ones-column (e14) matmul accumulating into
   row 13 of the same bias psum -> drained together, no extra drains.
 - bias + e13 matmuls issue as TWO concurrent 32-aligned column strips
   (M=14 at psum rows 0:14 and 32:46) -> ~2x PE throughput; the two strip
   partial-sums are merged once in the attention phase (cheap), not per panel.
 - x-path (LN, qkv, q/k LN, kT/qT) is sliced into small pieces interleaved
   into the pair-panel loop so PE/DVE/ACT stay busy while DMA streams pair.
"""

import sys

sys.path.insert(0, "/opt/trn_rl_repo")

from contextlib import ExitStack

import ml_dtypes
import numpy as np

import concourse.bass as bass
import concourse.tile as tile
from concourse import bacc, mybir
from concourse.bass_utils import run_bass_kernel_spmd
from concourse.masks import make_identity

F32 = mybir.dt.float32
FP8 = mybir.dt.float8e4
BF16 = mybir.dt.bfloat16
AF = mybir.ActivationFunctionType
OP = mybir.AluOpType

C = 768
H = 12
HD = 64
N = 512
NCORES = 8
IB = N // NCORES  # 64 i rows per core
NIJ = IB * N  # 32768 pair rows per core
NPNL = IB  # 64 panels: panel p = i-row p, all 512 j
TJ = 1024  # j columns per DMA tile (2 panels)
NT = NPNL // 2  # 32 dma tiles
EPS = 1e-5
RC = 1.0 / C

bf = ml_dtypes.bfloat16


def _build():
    nc = bacc.Bacc(
        "TRN2", target_bir_lowering=False, debug=False, num_devices=NCORES
    )

    pairT_d = nc.dram_tensor("pairT", [C, NIJ], BF16, kind="ExternalInput").ap()
    x_d = nc.dram_tensor("x_s", [N, C], BF16, kind="ExternalInput").ap()
    wqkvt_d = nc.dram_tensor("wqkvt", [C, 3 * C], BF16, kind="ExternalInput").ap()
    bqkv_d = nc.dram_tensor("bqkv", [1, 3 * C], BF16, kind="ExternalInput").ap()
    wprojt_d = nc.dram_tensor("wprojt", [C, C], BF16, kind="ExternalInput").ap()
    bproj_d = nc.dram_tensor("bproj", [1, C], BF16, kind="ExternalInput").ap()
    wg14_d = nc.dram_tensor("wg14", [C, 14], BF16, kind="ExternalInput").ap()
    ncs_d = nc.dram_tensor("ncs", [1, H], BF16, kind="ExternalInput").ap()
    reps_d = nc.dram_tensor("lnreps", [6, C], BF16, kind="ExternalInput").ap()
    out_d = nc.dram_tensor("out", [IB, C], F32, kind="ExternalOutput").ap()

    with tile.TileContext(nc) as tc, ExitStack() as ctx:
        sing = ctx.enter_context(tc.tile_pool(name="sing", bufs=1))
        pairp = ctx.enter_context(tc.tile_pool(name="pairp", bufs=3))
        sqp = ctx.enter_context(tc.tile_pool(name="sqp", bufs=2))
        statp = ctx.enter_context(tc.tile_pool(name="statp", bufs=2))
        stagep = ctx.enter_context(tc.tile_pool(name="stagep", bufs=2))
        attnp = ctx.enter_context(tc.tile_pool(name="attnp", bufs=2))
        ps_pair = ctx.enter_context(tc.tile_pool(name="ps_pair", bufs=4, space="PSUM"))
        ps_mm = ctx.enter_context(tc.tile_pool(name="ps_mm", bufs=2, space="PSUM"))
        ps_t = ctx.enter_context(tc.tile_pool(name="ps_t", bufs=2, space="PSUM"))

        # ---- singles ----
        id128 = sing.tile([128, 128], BF16)
        make_identity(nc, id128)
        ones_col = sing.tile([1, 128], BF16)
        nc.vector.memset(ones_col, 1.0)
        e14 = sing.tile([128, 14], BF16)
        nc.vector.memset(e14, 0.0)
        nc.vector.memset(e14[:, 13:14], 1.0)
        e14f8 = sing.tile([128, 14], FP8)
        nc.vector.memset(e14f8, 0.0)
        nc.vector.memset(e14f8[:, 13:14], 1.0)

        wg14 = sing.tile([128, 6, 14], BF16)
        nc.sync.dma_start(out=wg14, in_=wg14_d.rearrange("(k p) o -> p k o", p=128))

        # bias_r: [i, 28, j]; rows 0:14 strip-A partial, 14:28 strip-B partial
        # (rows within strip: 0-11 heads, 12 = 768*mu, 13 = sumsq).
        # merge_bias_half folds B into A in place: rows 0:14 become the total.
        bias_r = sing.tile([IB, 28, N], BF16)
        rmap = sing.tile([IB, N], BF16)
        ncsrep = sing.tile([IB, H], BF16)

        # x-path singles
        wqkvt = sing.tile([128, 6, 3 * C], BF16)
        bqkv = sing.tile([1, 3 * C], BF16)
        reps = sing.tile([128, 6, C], BF16)
        xn = sing.tile([128, 4, C], BF16)
        xnT = sing.tile([128, 6, N], BF16)
        qkv = sing.tile([128, 4, 3 * C], BF16)
        kT = sing.tile([128, 6, N], BF16)
        qT = sing.tile([128, 6, IB], BF16)
        wprojt = sing.tile([128, 6, C], BF16)
        bproj = sing.tile([1, C], BF16)

        wq_r = wqkvt_d.rearrange("(k p) o -> p k o", p=128)

        # ================== x-path pieces ==================
        xstat = {}

        def x_dma_initial():
            nc.sync.dma_start(out=xn, in_=x_d.rearrange("(t p) c -> p t c", p=128))
            nc.sync.dma_start(out=bqkv, in_=bqkv_d)
            for rI in range(6):
                nc.gpsimd.dma_start(
                    out=reps[:, rI, :],
                    in_=bass.AP(
                        tensor=reps_d.tensor, offset=rI * C, ap=[[0, 128], [1, C]]
                    ),
                )
            nc.gpsimd.dma_start(
                out=ncsrep,
                in_=bass.AP(tensor=ncs_d.tensor, offset=0, ap=[[0, IB], [1, H]]),
            )

        def x_ln_stats():
            bnx = statp.tile([128, 8, 6], F32, tag="bnx", bufs=1)
            mvx = statp.tile([128, 4, 2], F32, tag="mvx", bufs=1)
            rxx = statp.tile([128, 4], F32, tag="rx", bufs=1)
            xv = xn.rearrange("p t (k c) -> p t k c", k=2)
            for t in range(4):
                for k in range(2):
                    nc.vector.bn_stats(out=bnx[:, 2 * t + k, :], in_=xv[:, t, k, :])
            for t in range(4):
                nc.vector.bn_aggr(out=mvx[:, t, :], in_=bnx[:, 2 * t : 2 * t + 2, :])
            nc.vector.tensor_scalar(
                out=rxx, in0=mvx[:, :, 1], scalar1=EPS, scalar2=None, op0=OP.add
            )
            nc.vector.reciprocal(out=rxx, in_=rxx)
            nc.scalar.activation(out=rxx, in_=rxx, func=AF.Sqrt)
            xstat["mvx"] = mvx
            xstat["rxx"] = rxx

        def x_ln_apply(ts0, ts1):
            mvx, rxx = xstat["mvx"], xstat["rxx"]
            for t in (ts0, ts1):
                nc.vector.tensor_scalar(
                    out=xn[:, t, :], in0=xn[:, t, :],
                    scalar1=mvx[:, t, 0:1], scalar2=rxx[:, t : t + 1],
                    op0=OP.subtract, op1=OP.mult,
                )
                nc.vector.tensor_tensor(
                    out=xn[:, t, :], in0=xn[:, t, :], in1=reps[:, 0, :], op=OP.mult
                )
                nc.vector.tensor_tensor(
                    out=xn[:, t, :], in0=xn[:, t, :], in1=reps[:, 1, :], op=OP.add
                )

        def x_transpose(ch):
            pst = ps_t.tile([128, N], BF16, tag="pst")
            for t in range(4):
                nc.tensor.transpose(
                    pst[:, t * 128 : (t + 1) * 128],
                    xn[:, t, ch * 128 : (ch + 1) * 128],
                    id128,
                )
            nc.vector.tensor_copy(out=xnT[:, ch, :], in_=pst)

        QKV_OCH = [(0, 512), (512, 512), (1024, 512), (1536, 512), (2048, 256)]

        def x_qkv_piece(idx):
            t, oi = idx // 5, idx % 5
            occ, ocs = QKV_OCH[oi]
            pmm = ps_mm.tile([128, N], F32, tag="mm")
            for ch in range(6):
                nc.tensor.matmul(
                    pmm[:, 0:ocs],
                    lhsT=xnT[:, ch, t * 128 : (t + 1) * 128],
                    rhs=wqkvt[:, ch, occ : occ + ocs],
                    start=(ch == 0), stop=False,
                )
            nc.tensor.matmul(
                pmm[:, 0:ocs], lhsT=ones_col[:, 0:128],
                rhs=bqkv[:, occ : occ + ocs], start=False, stop=True,
            )
            if idx % 2 == 0:
                nc.vector.tensor_copy(out=qkv[:, t, occ : occ + ocs], in_=pmm[:, 0:ocs])
            else:
                nc.scalar.copy(out=qkv[:, t, occ : occ + ocs], in_=pmm[:, 0:ocs])

        def qk_ln_stats1():
            bnq = statp.tile([128, 16, 6], F32, tag="bnq", bufs=1)
            qv = qkv[:, :, 0 : 2 * C].rearrange("p t (k c) -> p t k c", k=4)
            for t in range(4):
                for k in range(4):
                    nc.vector.bn_stats(out=bnq[:, 4 * t + k, :], in_=qv[:, t, k, :])
            xstat["bnq"] = bnq

        def qk_ln_stats2():
            bnq = xstat["bnq"]
            mvq = statp.tile([128, 8, 2], F32, tag="mvq", bufs=1)
            rqq = statp.tile([128, 8], F32, tag="rq", bufs=1)
            for col in range(8):
                nc.vector.bn_aggr(out=mvq[:, col, :], in_=bnq[:, 2 * col : 2 * col + 2, :])
            nc.vector.tensor_scalar(
                out=rqq, in0=mvq[:, :, 1], scalar1=EPS, scalar2=None, op0=OP.add
            )
            nc.vector.reciprocal(out=rqq, in_=rqq)
            nc.scalar.activation(out=rqq, in_=rqq, func=AF.Sqrt)
            xstat["mvq"] = mvq
            xstat["rqq"] = rqq

        def qk_ln_apply(t, qi):
            mvq, rqq = xstat["mvq"], xstat["rqq"]
            off = qi * C
            col = t * 2 + qi
            gr = 2 + 2 * qi
            nc.vector.tensor_scalar(
                out=qkv[:, t, off : off + C], in0=qkv[:, t, off : off + C],
                scalar1=mvq[:, col, 0:1], scalar2=rqq[:, col : col + 1],
                op0=OP.subtract, op1=OP.mult,
            )
            nc.vector.tensor_tensor(
                out=qkv[:, t, off : off + C], in0=qkv[:, t, off : off + C],
                in1=reps[:, gr, :], op=OP.mult,
            )
            nc.vector.tensor_tensor(
                out=qkv[:, t, off : off + C], in0=qkv[:, t, off : off + C],
                in1=reps[:, gr + 1, :], op=OP.add,
            )

        def k_transpose(ch):
            pst = ps_t.tile([128, N], BF16, tag="pst")
            for t in range(4):
                nc.tensor.transpose(
                    pst[:, t * 128 : (t + 1) * 128],
                    qkv[:, t, C + ch * 128 : C + (ch + 1) * 128],
                    id128,
                )
            nc.vector.tensor_copy(out=kT[:, ch, :], in_=pst)

        def q_transpose():
            pst = ps_t.tile([128, N], BF16, tag="pst")
            for ch in range(6):
                nc.tensor.transpose(
                    pst[:, ch * IB : (ch + 1) * IB],
                    qkv[0:IB, 0, ch * 128 : (ch + 1) * 128],
                    id128[0:IB, 0:IB],
                )
            nc.vector.tensor_copy(
                out=qT.rearrange("p a b -> p (a b)"), in_=pst[:, 0 : 6 * IB]
            )

        # panel -> list of closures
        sched = {}

        def at(p, fn):
            sched.setdefault(p, []).append(fn)

        at(2, x_ln_stats)
        at(3, lambda: x_ln_apply(0, 1))
        at(4, lambda: x_ln_apply(2, 3))
        for ch in range(6):
            at(5 + ch, lambda ch=ch: x_transpose(ch))
        for idx in range(20):
            at(32 + idx, lambda idx=idx: x_qkv_piece(idx))
        at(53, qk_ln_stats1)
        at(54, qk_ln_stats2)
        for i, (t, qi) in enumerate([(t, qi) for t in range(4) for qi in range(2)]):
            at(55 + i // 2, lambda t=t, qi=qi: qk_ln_apply(t, qi))
        at(59, lambda: k_transpose(0))
        at(60, lambda: k_transpose(1))
        at(60, lambda: k_transpose(2))
        at(61, lambda: k_transpose(3))
        at(61, lambda: k_transpose(4))
        at(62, lambda: k_transpose(5))
        at(63, q_transpose)

        # ================== maps ==================
        va = sing.tile([IB, N], F32)
        tsm = sing.tile([IB, N], BF16)

        def merge_bias_half(b):
            r = slice(32 * b, 32 * b + 32)
            nc.vector.tensor_tensor(
                out=bias_r[r, 0:14, :].rearrange("p a b -> p (a b)"),
                in0=bias_r[r, 0:14, :].rearrange("p a b -> p (a b)"),
                in1=bias_r[r, 14:28, :].rearrange("p a b -> p (a b)"),
                op=OP.add,
            )

        def emit_maps_half(b):
            r = slice(32 * b, 32 * b + 32)
            # merged bias_r row 12 = 768*mu, row 13 = sumsq
            nc.vector.tensor_scalar(
                out=tsm[r], in0=bias_r[r, 12, :], scalar1=RC, scalar2=None, op0=OP.mult
            )
            nc.gpsimd.tensor_tensor(out=tsm[r], in0=tsm[r], in1=tsm[r], op=OP.mult)
            nc.vector.scalar_tensor_tensor(
                out=va[r], in0=bias_r[r, 13, :], scalar=RC,
                in1=tsm[r], op0=OP.mult, op1=OP.subtract,
            )
            nc.vector.tensor_scalar(
                out=va[r], in0=va[r], scalar1=EPS, scalar2=None, op0=OP.add
            )
            nc.vector.reciprocal(out=va[r], in_=va[r])
            nc.scalar.activation(out=rmap[r], in_=va[r], func=AF.Sqrt)

        # ================== pair panel loop ==================
        pv = pairT_d.rearrange("(k p) (n j) -> n p k j", p=128, j=TJ)
        x_dma_initial()

        STRIP_CH = ((0, 2, 4), (1, 3, 5))  # chunks per column strip

        # panels whose psum still needs draining: list of (bps, pnl)
        pending = []

        def drain_pending():
            # deps (the panels' matmuls) completed a tile ago, so these
            # copies never stall the ACT/DVE queues
            while pending:
                bps_p, pnl_p = pending.pop(0)
                stg = stagep.tile([46, N], BF16, tag="stg")
                if pnl_p % 2 == 0:
                    nc.scalar.copy(out=stg, in_=bps_p[0:46, :])
                else:
                    nc.vector.tensor_copy(out=stg, in_=bps_p[0:46, :])
                nc.gpsimd.dma_start(
                    out=bias_r[pnl_p : pnl_p + 1, 0:14, :], in_=stg[0:14, :]
                )
                nc.gpsimd.dma_start(
                    out=bias_r[pnl_p : pnl_p + 1, 14:28, :], in_=stg[32:46, :]
                )

        for ti in range(NT):
            grp = pairp.tile([128, 6, TJ], BF16, tag="grp")
            nc.sync.dma_start(out=grp, in_=pv[ti])
            if 1 <= ti <= 6:
                k = ti - 1
                nc.sync.dma_start(out=wqkvt[:, k, :], in_=wq_r[:, k, :])
            if ti == 7:
                nc.sync.dma_start(
                    out=wprojt, in_=wprojt_d.rearrange("(k p) o -> p k o", p=128)
                )
                nc.sync.dma_start(out=bproj, in_=bproj_d)
            drain_pending()

            # squares: DVE chunks 0:3 (bf16), ACT chunks 3:6 (fp8 out)
            sqD = sqp.tile([128, 3, TJ], BF16, tag="sqD")
            nc.vector.tensor_tensor(
                out=sqD.rearrange("p a b -> p (a b)"),
                in0=grp[:, 0:3, :].rearrange("p a b -> p (a b)"),
                in1=grp[:, 0:3, :].rearrange("p a b -> p (a b)"),
                op=OP.mult,
            )
            sqA = sqp.tile([128, 3, TJ], FP8, tag="sqA")
            nc.scalar.activation(
                out=sqA.rearrange("p a b -> p (a b)"),
                in_=grp[:, 3:6, :].rearrange("p a b -> p (a b)"),
                func=AF.Square,
            )
            # DVE partial: aD2 = sqD0 + sqD1 + sqD2 (second add in place)
            aD2 = sqp.tile([128, TJ], BF16, tag="aD2")
            nc.vector.tensor_tensor(
                out=aD2, in0=sqD[:, 0, :], in1=sqD[:, 1, :], op=OP.add
            )
            nc.vector.tensor_tensor(
                out=aD2, in0=aD2, in1=sqD[:, 2, :], op=OP.add
            )

            # bias matmuls for both panels first (dep: grp DMA only), then
            # the e13 sumsq rows (dep: squares) -- keeps PE streaming while
            # the squares are still in flight on DVE/ACT
            bps2 = []
            for s in range(2):
                js = slice(s * N, (s + 1) * N)
                bps = ps_pair.tile([128, N], F32, tag="bps")
                bps2.append(bps)
                # two concurrent column strips (psum rows 0:14 / 32:46),
                # interleaved in program order so they stream concurrently
                for cc in range(3):
                    for g, chs in enumerate(STRIP_CH):
                        nc.tensor.matmul(
                            bps[32 * g : 32 * g + 14, :],
                            lhsT=wg14[:, chs[cc], :],
                            rhs=grp[:, chs[cc], js],
                            start=(cc == 0), stop=False,
                        )
            for s in range(2):
                pnl = 2 * ti + s
                js = slice(s * N, (s + 1) * N)
                bps = bps2[s]
                # sumsq rows: strip A gets aD2 + sqA0; strip B gets sqA1 + sqA2
                nc.tensor.matmul(
                    bps[0:14, :], lhsT=e14, rhs=aD2[:, js],
                    start=False, stop=False,
                )
                nc.tensor.matmul(
                    bps[32:46, :], lhsT=e14f8, rhs=sqA[:, 1, js],
                    start=False, stop=False,
                )
                nc.tensor.matmul(
                    bps[0:14, :], lhsT=e14f8, rhs=sqA[:, 0, js],
                    start=False, stop=True,
                )
                nc.tensor.matmul(
                    bps[32:46, :], lhsT=e14f8, rhs=sqA[:, 2, js],
                    start=False, stop=True,
                )
                pending.append((bps, pnl))

                for fn in sched.get(pnl, ()):
                    fn()
                if pnl == 34:
                    merge_bias_half(0)
                    emit_maps_half(0)

        drain_pending()
        merge_bias_half(1)
        emit_maps_half(1)

        # ================== attention ==================
        o_sb = sing.tile([IB, H, HD], BF16)
        bfull = sing.tile([IB, H, N], BF16)

        def emit_bfix(h):
            t1 = attnp.tile([IB, N], BF16, tag="t1")
            nc.vector.scalar_tensor_tensor(
                out=t1, in0=bias_r[:, 12, :], scalar=ncsrep[:, h : h + 1],
                in1=bias_r[:, h, :], op0=OP.mult, op1=OP.add,
            )
            nc.gpsimd.tensor_tensor(
                out=bfull[:, h, :], in0=t1, in1=rmap, op=OP.mult
            )

        def emit_qk(h):
            sps = ps_pair.tile([IB, N], F32, tag="bps")
            bp = (h % 2) * 64
            nc.tensor.matmul(
                sps[0:IB, :],
                lhsT=qT[bp : bp + 64, h // 2, :],
                rhs=kT[bp : bp + 64, h // 2, :],
                start=True, stop=True,
            )
            return sps

        emit_bfix(0)
        emit_bfix(1)
        sps_next = emit_qk(0)
        pps_o0 = ps_mm.tile([128, N], F32, tag="mm")
        pps_o1 = ps_mm.tile([128, N], F32, tag="mm")
        pps_out = [pps_o0, pps_o1]
        for h in range(H):
            if h + 2 < H:
                emit_bfix(h + 2)
            sps = sps_next
            if h + 1 < H:
                sps_next = emit_qk(h + 1)
            sim = attnp.tile([IB, N], F32, tag="sim")
            nc.vector.scalar_tensor_tensor(
                out=sim, in0=sps[0:IB, :], scalar=0.125, in1=bfull[:, h, :],
                op0=OP.mult, op1=OP.add,
            )
            # logits are small (LN'd q/k, tiny weights): exp without max-sub
            den = attnp.tile([IB, 1], F32, tag="den")
            nc.scalar.activation(out=sim, in_=sim, func=AF.Exp, accum_out=den)
            nc.vector.reciprocal(out=den, in_=den)
            attn = attnp.tile([IB, N], BF16, tag="attn")
            nc.vector.tensor_scalar(
                out=attn, in0=sim, scalar1=den, scalar2=None, op0=OP.mult
            )
            aps = ps_t.tile([128, N], BF16, tag="pst")
            for jc in range(4):
                nc.tensor.transpose(
                    aps[:, jc * IB : (jc + 1) * IB],
                    attn[:, jc * 128 : (jc + 1) * 128],
                    id128[0:IB, 0:IB],
                )
            aT = attnp.tile([128, 4, IB], BF16, tag="aT")
            nc.scalar.copy(
                out=aT.rearrange("p a b -> p (a b)"), in_=aps[:, 0 : 4 * IB]
            )
            ops = ps_t.tile([IB, N], F32, tag="pst")
            for jc in range(4):
                nc.tensor.matmul(
                    ops[0:IB, 0:HD],
                    lhsT=aT[:, jc, :],
                    rhs=qkv[:, jc, 2 * C + h * HD : 2 * C + (h + 1) * HD],
                    start=(jc == 0), stop=(jc == 3),
                )
            nc.scalar.copy(out=o_sb[:, h, :], in_=ops[0:IB, 0:HD])

            # proj c-block ch = heads (2ch, 2ch+1): interleave into head loop
            if h % 2 == 1:
                ch = h // 2
                pso = ps_t.tile([128, N], BF16, tag="pst")
                nc.tensor.transpose(
                    pso[:, 0:IB],
                    o_sb.rearrange("p a b -> p (a b)")[:, ch * 128 : (ch + 1) * 128],
                    id128[0:IB, 0:IB],
                )
                oTc = attnp.tile([128, IB], BF16, tag="oTc")
                nc.scalar.copy(out=oTc, in_=pso[:, 0:IB])
                for oi, (occ, ocs) in enumerate([(0, 512), (512, 256)]):
                    nc.tensor.matmul(
                        pps_out[oi][0:IB, 0:ocs],
                        lhsT=oTc[:, 0:IB],
                        rhs=wprojt[:, ch, occ : occ + ocs],
                        start=(ch == 0), stop=False,
                    )

        # ================== output proj epilogue ==================
        out_sb = sing.tile([IB, C], F32)
        for oi, (occ, ocs) in enumerate([(0, 512), (512, 256)]):
            nc.tensor.matmul(
                pps_out[oi][0:IB, 0:ocs], lhsT=ones_col[:, 0:IB],
                rhs=bproj[:, occ : occ + ocs], start=False, stop=True,
            )
            nc.vector.tensor_copy(
                out=out_sb[:, occ : occ + ocs], in_=pps_out[oi][0:IB, 0:ocs]
            )
        nc.sync.dma_start(out=out_d, in_=out_sb)

    nc.compile()
    return nc


_NC = None
_LAST_MAPS = None


def prep_maps(x, pair, ln_g, ln_b, w_qkv, b_qkv, w_proj, b_proj, w_bias,
              pn_g, pn_b, qln_g, qln_b, kln_g, kln_b):
    x = np.asarray(x, np.float32)
    pair = np.asarray(pair, np.float32)
    wqkvt = np.ascontiguousarray(np.asarray(w_qkv, np.float32).T).astype(bf)
    wprojt = np.ascontiguousarray(np.asarray(w_proj, np.float32).T).astype(bf)
    wg_host = np.ascontiguousarray(
        (np.asarray(pn_g, np.float32)[:, None] * np.asarray(w_bias, np.float32).T)
    )
    cs = wg_host.sum(axis=0)  # colsum over c, [H]
    wg14 = np.concatenate(
        [wg_host, np.ones((C, 1), np.float32), np.zeros((C, 1), np.float32)],
        axis=1,
    ).astype(bf)  # [C, 14]; col 12 harvests 768*mu, col 13 = sumsq slot
    ncs = (-cs / C)[None].astype(bf)  # [1, H]; scaled: mu arrives as 768*mu
    reps = np.stack(
        [np.asarray(a, np.float32) for a in (ln_g, ln_b, qln_g, qln_b, kln_g, kln_b)]
    ).astype(bf)
    bqkv = np.asarray(b_qkv, np.float32)[None].astype(bf)
    bproj = np.asarray(b_proj, np.float32)[None].astype(bf)

    in_maps = []
    for k in range(NCORES):
        ps = pair[0, k * IB : (k + 1) * IB]  # [64, 512, 768]
        ps = np.roll(ps, -k * IB, axis=1)  # roll j to match rolled x
        pT = np.ascontiguousarray(ps.reshape(NIJ, C).T).astype(bf)  # [C, NIJ]
        xk = np.roll(x[0], -k * IB, axis=0).astype(bf)
        in_maps.append(
            {
                "pairT": pT,
                "x_s": np.ascontiguousarray(xk),
                "wqkvt": wqkvt,
                "bqkv": bqkv,
                "wprojt": wprojt,
                "bproj": bproj,
                "wg14": wg14,
                "ncs": ncs,
                "lnreps": reps,
            }
        )

    return in_maps


def kernel(**inputs):
    global _NC, _LAST_MAPS
    if _NC is None:
        _NC = _build()
    in_maps = prep_maps(**inputs)
    _LAST_MAPS = in_maps
    res = run_bass_kernel_spmd(_NC, in_maps, list(range(NCORES)))
    outs = [res.results[k]["out"] for k in range(NCORES)]
    return np.concatenate(outs, axis=0)[None].astype(np.float32)
